# revision 65
# baseline (speedup 1.0000x reference)
"""Multi-head attention Trainium2 kernel (v2).

Full inputs -> shard over 8 NeuronCores (batch x head-group) -> full output.

Per core c: batch b = c // 2, head-group hg = c % 2 (8 of 16 heads).
Column-shard Wq/Wk/Wv, row-shard Wo; each core computes a partial output
projection for its batch; host sums the two partials per batch and adds bo.

v2 layout/schedule (per core):
  - scores^T [k, q] with two heads of a pair packed on PE row halves
    (concurrent K=64 matmuls); exp on ScalarE only; mask multiply batched
    per kt-pair on DVE at 2x 16-bit rate.
  - V staged as [seq, 8 heads x 68] fp16 blocks: cols 0-63 = V, col 64 =
    ones (emits softmax denominators through the attn@V matmul), 65-67 pad
    for 4B alignment.  attn@V accumulates ctx^T rows 0-64 in PSUM.
  - normalization: reciprocal_approx_fast on PSUM row 64 -> gpsimd
    partition_broadcast -> one DVE multiply into ctxT.
  - schedule: K-proj, Q-proj, then attention starts; V-proj woven into
    q-chunk 0's score stream; output projection folded into each q-chunk.
  - PSUM: proj/scores pool 2x[128,1024] (4 banks) + ctx 2x[128,512]
    (2 banks) + out-proj 1x[128,1024] (2 banks) = 8 banks.
"""

import os
import sys

for _p in ("/opt/trn_rl_repo", "/root/.axon_site/_ro/trn_rl_repo"):
    if os.path.isdir(_p) and _p not in sys.path:
        sys.path.insert(0, _p)

import numpy as np
import ml_dtypes

B, S, D, H = 4, 2048, 1024, 16
DK = 64
N_CORES = 8
HG = 2                  # head groups (cores per batch)
DH = D // HG            # 512: d_out per core
QC = 512                # q-chunk width per score matmul (one PSUM bank)
VB = 68                 # va block stride (64 vals + ones col + 3 pad)


def build_attention_nc(s=S, d=D, dh=DH, qc=QC):
    """Build the single-core Bass program (SPMD across 8 cores)."""
    import concourse.mybir as mybir
    import concourse.tile as tile
    from concourse import bacc

    f32 = mybir.dt.float32
    f16 = mybir.dt.float16
    bf16 = mybir.dt.bfloat16
    EXPF = mybir.ActivationFunctionType.Exp

    n_h = dh // DK            # heads on this core (8)
    n_hp = n_h // 2           # head pairs (4)
    n_di = d // 128           # d_model 128-tiles (8)
    n_do = dh // 128          # d_out 128-tiles (4) == head pairs
    n_kt = s // 128           # key 128-tiles (16)
    n_qc = s // qc            # q chunks (4)
    n_st = s // 128           # seq 128-tiles (16)
    VA = n_h * VB             # va width per seq-tile

    nc = bacc.Bacc(None, target_bir_lowering=False)

    xqT = nc.dram_tensor("xqT", [d, s], f16, kind="ExternalInput")
    xkT = nc.dram_tensor("xkT", [d, s], f16, kind="ExternalInput")
    xvT = nc.dram_tensor("xvT", [d, s], f16, kind="ExternalInput")
    maskT = nc.dram_tensor("maskT", [s, s], f16, kind="ExternalInput")
    wq = nc.dram_tensor("wq", [d, dh], f16, kind="ExternalInput")
    wk = nc.dram_tensor("wk", [d, dh], f16, kind="ExternalInput")
    wv = nc.dram_tensor("wv", [d, dh], f16, kind="ExternalInput")
    wo = nc.dram_tensor("wo", [dh, d], f16, kind="ExternalInput")
    # bqT/bkT: [128, n_do] (column do = bias slice) for per-partition adds
    bqT = nc.dram_tensor("bqT", [128, n_do], f32, kind="ExternalInput")
    bkT = nc.dram_tensor("bkT", [128, n_do], f32, kind="ExternalInput")
    bv = nc.dram_tensor("bv", [1, dh], bf16, kind="ExternalInput")
    ones_d = nc.dram_tensor("ones_d", [1, 512], bf16, kind="ExternalInput")
    oT = nc.dram_tensor("oT", [d, s], f32, kind="ExternalOutput")

    # mask viewed as [p, kt, q] so one DMA grabs a [128, 8, qc] half-chunk
    maskT3 = maskT.rearrange("(kt p) q -> p kt q", p=128)

    scale = float(1.0 / np.sqrt(np.float32(DK)))

    with tile.TileContext(nc) as tc:
        with (
            tc.tile_pool(name="stage", bufs=16) as stage_pool,
            tc.tile_pool(name="w", bufs=18) as w_pool,
            tc.tile_pool(name="wo", bufs=n_do) as wo_pool,
            tc.tile_pool(name="qk", bufs=2 * n_do) as qk_pool,
            tc.tile_pool(name="va", bufs=n_st) as va_pool,
            tc.tile_pool(name="ctxT", bufs=n_do) as ctxT_pool,
            tc.tile_pool(name="mask", bufs=4) as mask_pool,
            tc.tile_pool(name="e", bufs=5) as e_pool,
            tc.tile_pool(name="nrm", bufs=1) as nrm_pool,
            tc.tile_pool(name="nrmbc", bufs=2) as nrmbc_pool,
            tc.tile_pool(name="osb", bufs=2) as osb_pool,
            tc.tile_pool(name="const", bufs=1) as const_pool,
            tc.tile_pool(name="ps", bufs=2, space="PSUM") as ps_pool,
            tc.tile_pool(name="cps", bufs=3, space="PSUM") as c_pool,
            tc.tile_pool(name="ops", bufs=1, space="PSUM") as o_pool,
        ):
            # ---------------- constants ---------------------------------
            ones = const_pool.tile([1, 128], bf16, tag="ones", name="ones")
            nc.sync.dma_start(ones[:, :], ones_d[:, 0:128])
            bqT_sb = const_pool.tile([128, n_do], f32, tag="biasq", name="bqT_sb")
            bkT_sb = const_pool.tile([128, n_do], f32, tag="biask", name="bkT_sb")
            bv_sb = const_pool.tile([1, dh], bf16, tag="biasv", name="bv_sb")
            nc.sync.dma_start(bqT_sb[:, :], bqT[:, :])
            nc.sync.dma_start(bkT_sb[:, :], bkT[:, :])
            nc.sync.dma_start(bv_sb[:, :], bv[:, :])

            # ---------------- input staging (pool-gated prefetch) -------
            def stage_x(xdram):
                xts = []
                for di in range(n_di):
                    xt = stage_pool.tile([128, s], f16, tag="x", name="xt")
                    nc.sync.dma_start(xt[:, :], xdram[di * 128:(di + 1) * 128, :])
                    xts.append(xt)
                return xts

            def stage_w(wdram):
                wts = []
                for di in range(n_di):
                    wt = w_pool.tile([128, dh], f16, tag="w", name="wt")
                    nc.sync.dma_start(wt[:, :], wdram[di * 128:(di + 1) * 128, :])
                    wts.append(wt)
                return wts

            # interleave wv/xv DMAs so V-proj's di-matmuls can start as soon
            # as each (wv[di], xv[di]) pair lands, not after the full wv set
            wv_t, xv_t = [], []
            for di in range(n_di):
                wt = w_pool.tile([128, dh], f16, tag="w", name="wt")
                nc.sync.dma_start(wt[:, :], wv[di * 128:(di + 1) * 128, :])
                wv_t.append(wt)
                xt = stage_pool.tile([128, s], f16, tag="x", name="xt")
                nc.sync.dma_start(xt[:, :], xvT[di * 128:(di + 1) * 128, :])
                xv_t.append(xt)
            wk_t = stage_w(wk)
            xk_t = stage_x(xkT)
            wq_t = stage_w(wq)
            xq_t = stage_x(xqT)
            wo_tiles = []
            for t in range(n_do):
                wt = wo_pool.tile([128, d], f16, tag="wo", name="wot")
                nc.sync.dma_start(wt[:, :], wo[t * 128:(t + 1) * 128, :])
                wo_tiles.append(wt)

            # ---------------- K / Q projections --------------------------
            def proj_kq(wts, xts, bsb, outs, do, scps=(0, 1)):
                ot = outs[do]
                for scp in scps:
                    ps = ps_pool.tile([128, 1024], f32, tag="ps", name="ps")
                    for half in range(2):
                        sc = scp * 2 + half
                        for di in range(n_di):
                            nc.tensor.matmul(
                                ps[:, half * 512:(half + 1) * 512],
                                wts[di][:, do * 128:(do + 1) * 128],
                                xts[di][:, sc * 512:(sc + 1) * 512],
                                start=(di == 0), stop=(di == n_di - 1),
                            )
                    nc.vector.tensor_scalar_add(
                        ot[:, scp * 1024:(scp + 1) * 1024],
                        ps[:, :], bsb[:, do:do + 1])

            kT = [qk_pool.tile([128, s], bf16, tag="qk", name=f"kT{t}")
                  for t in range(n_do)]
            qT = [qk_pool.tile([128, s], bf16, tag="qk", name=f"qT{t}")
                  for t in range(n_do)]

            # ---------------- V projection (emitted via closure) ---------
            va_tiles = [None] * n_st

            def emit_vproj(st_lo, st_hi):
                for stp in range(st_lo // 2, st_hi // 2):
                    vp = ps_pool.tile([128, 1024], f32, tag="ps", name="vp")
                    for half in range(2):
                        st = stp * 2 + half
                        for di in range(n_di):
                            nc.tensor.matmul(
                                vp[:, half * 512:(half + 1) * 512],
                                xv_t[di][:, st * 128:(st + 1) * 128],
                                wv_t[di][:, :],
                                start=(di == 0), stop=False,
                            )
                        nc.tensor.matmul(
                            vp[:, half * 512:(half + 1) * 512],
                            ones[:, 0:128], bv_sb[:, :],
                            start=False, stop=True,
                        )
                    for half in range(2):
                        st = stp * 2 + half
                        va = va_pool.tile([128, VA], f16, tag="va", name="va")
                        va3 = va.rearrange("p (h x) -> p h x", x=VB)
                        nc.vector.tensor_copy(
                            va3[:, :, 0:64],
                            vp[:, half * 512:(half + 1) * 512]
                            .rearrange("p (h x) -> p h x", x=64),
                        )
                        nc.gpsimd.memset(va3[:, :, 64:65], 1.0)
                        va_tiles[st] = va

            # ---------------- attention emission helpers -----------------
            mask_tiles = {}   # (qcb, quarter) -> tile [128, 4, qc]

            def emit_mask_dma(qcb):
                for quarter in range(4):
                    mt = mask_pool.tile([128, 4, qc], f16, tag="m", name="mt")
                    nc.sync.dma_start(
                        mt[:, :, :],
                        maskT3[:, quarter * 4:(quarter + 1) * 4,
                               qcb * qc:(qcb + 1) * qc],
                    )
                    mask_tiles[(qcb, quarter)] = mt

            pt_tiles = {}     # (qcb, hp, pair) -> masked-prob tile

            def emit_scores(qcb, hp, filler=None):
                for pair in range(n_kt // 2):
                    if pair == 2 and filler is not None:
                        filler()
                    et = e_pool.tile([128, 2 * 1024], f16, tag="e", name="et")
                    for j in range(2):
                        kt = pair * 2 + j
                        sp = ps_pool.tile([128, 1024], f32, tag="ps", name="sp")
                        for hh in range(2):
                            lo = hh * 64
                            nc.tensor.matmul(
                                sp[:, hh * qc:(hh + 1) * qc],
                                kT[hp][lo:lo + 64, kt * 128:(kt + 1) * 128],
                                qT[hp][lo:lo + 64, qcb * qc:(qcb + 1) * qc],
                                start=True, stop=True,
                            )
                        nc.scalar.activation(
                            et[:, j * 1024:(j + 1) * 1024], sp[:, :], EXPF,
                            scale=scale)
                    mt = mask_tiles[(qcb, pair // 2)]
                    m4 = (mt[:, (pair % 2) * 2:(pair % 2) * 2 + 2, :]
                          .unsqueeze(2).broadcast_to([128, 2, 2, qc]))
                    nc.vector.tensor_mul(
                        et[:, :].rearrange("p (k h q) -> p k h q", k=2, q=qc),
                        et[:, :].rearrange("p (k h q) -> p k h q", k=2, q=qc),
                        m4)
                    pt_tiles[(qcb, hp, pair)] = et

            def emit_attnv_norm(qcb, hp):
                cps = [c_pool.tile([128, qc], f32, tag="c", name="cp")
                       for _ in range(2)]
                for pair in range(n_kt // 2):
                    pt = pt_tiles.pop((qcb, hp, pair))
                    for j in range(2):
                        kt = pair * 2 + j
                        for hh in range(2):
                            h = hp * 2 + hh
                            nc.tensor.matmul(
                                cps[hh][0:65, :],
                                va_tiles[kt][:, h * VB:h * VB + 65],
                                pt[:, j * 1024 + hh * qc:
                                   j * 1024 + (hh + 1) * qc],
                                start=(kt == 0), stop=(kt == n_kt - 1),
                                skip_group_check=True,
                            )
                # normalize: ctx[0:64] * (1 / ctx[64]) -> ctxT
                for hh in range(2):
                    lt = nrm_pool.tile([1, qc], f32, tag="l", name="lt")
                    nc.vector.tensor_copy(lt[0:1, :], cps[hh][64:65, :])
                    rt = nrm_pool.tile([1, qc], f32, tag="r", name="rt")
                    nc.vector.reciprocal_approx_fast(rt[0:1, :], lt[0:1, :])
                    bc = nrmbc_pool.tile([64, qc], f32, tag="bc", name="bc")
                    nc.gpsimd.partition_broadcast(bc[:, :], rt[0:1, :], 64)
                    nc.vector.tensor_mul(
                        ctxT[hp][hh * 64:hh * 64 + 64, qcb * qc:(qcb + 1) * qc],
                        cps[hh][0:64, :], bc[:, :])

            def emit_outproj(qcb, dm_lo=0, dm_hi=None, pool=None):
                for dm in range(dm_lo, n_di if dm_hi is None else dm_hi):
                    op = (pool or o_pool).tile([128, 512], f32,
                                               tag="c" if pool else "o",
                                               name="ops")
                    for t in range(n_do):
                        nc.tensor.matmul(
                            op[:, :],
                            wo_tiles[t][:, dm * 128:(dm + 1) * 128],
                            ctxT[t][:, qcb * qc:(qcb + 1) * qc],
                            start=(t == 0), stop=(t == n_do - 1),
                        )
                    osb = osb_pool.tile([128, 512], f32, tag="osb", name="osb")
                    nc.vector.tensor_copy(osb[:, :], op[:, :])
                    nc.sync.dma_start(
                        oT[dm * 128:(dm + 1) * 128, qcb * qc:(qcb + 1) * qc],
                        osb[:, :],
                    )

            ctxT = [ctxT_pool.tile([128, s], f16, tag="ctxT", name=f"ctxT{t}")
                    for t in range(n_do)]

            # ---------------- schedule -----------------------------------
            # V-proj first, then K0/Q0 so attention streams early; the
            # remaining K/Q projections fill PE gaps during qcb0.
            emit_vproj(0, n_st)
            proj_kq(wk_t, xk_t, bkT_sb, kT, 0)
            proj_kq(wq_t, xq_t, bqT_sb, qT, 0)
            emit_mask_dma(0)
            emit_scores(0, 0)
            # qcb0 scores only read the scp0 half of qT, so Q[hp] scp1 is
            # deferred into qcb1's stream; K[hp] scp1 is woven between score
            # pairs (needed from pair 4 on) to avoid 14us projection blocks.
            for hp in range(1, n_hp):
                proj_kq(wk_t, xk_t, bkT_sb, kT, hp, scps=(0,))
                proj_kq(wq_t, xq_t, bqT_sb, qT, hp, scps=(0,))
                emit_attnv_norm(0, hp - 1)
                emit_scores(0, hp, filler=lambda h=hp: proj_kq(
                    wk_t, xk_t, bkT_sb, kT, h, scps=(1,)))
            emit_attnv_norm(0, n_hp - 1)
            # out-proj for qcb q is spread across qcb q+1's attention stream
            # (2 dm-groups per hp) so its single-bank evac stalls hide under
            # attention matmuls instead of idling the PE at qcb boundaries.
            for qcb in range(1, n_qc):
                emit_mask_dma(qcb)
                for hp in range(n_hp):
                    if qcb == 1 and hp >= 1:
                        proj_kq(wq_t, xq_t, bqT_sb, qT, hp, scps=(1,))
                    emit_scores(qcb, hp)
                    emit_outproj(qcb - 1, hp * 2, hp * 2 + 2)
                    emit_attnv_norm(qcb, hp)
            # final qcb's out-proj: attention is done, so rotate through the
            # (now idle) cps ring for pipelined evacuation.  Each dm group's
            # t=0..2 matmuls only need the first three head-pairs' ctxT, so
            # they run while the last head-pair's normalization drains; the
            # t=3 closer + evac follows once three groups are in flight.
            qcb = n_qc - 1

            def close_dm(dm, op):
                nc.tensor.matmul(
                    op[:, :],
                    wo_tiles[n_do - 1][:, dm * 128:(dm + 1) * 128],
                    ctxT[n_do - 1][:, qcb * qc:(qcb + 1) * qc],
                    start=False, stop=True,
                )
                osb = osb_pool.tile([128, 512], f32, tag="osb", name="osb")
                nc.vector.tensor_copy(osb[:, :], op[:, :])
                nc.sync.dma_start(
                    oT[dm * 128:(dm + 1) * 128, qcb * qc:(qcb + 1) * qc],
                    osb[:, :],
                )

            pend = []
            for dm in range(n_di):
                if len(pend) == 3:
                    close_dm(*pend.pop(0))
                # dm0 uses the free o-bank: its alloc would otherwise wait on
                # the cps slot released only by the last norm's multiply
                op = (o_pool if dm == 0 else c_pool).tile(
                    [128, 512], f32, tag="o" if dm == 0 else "c", name="ops")
                for t in range(n_do - 1):
                    nc.tensor.matmul(
                        op[:, :],
                        wo_tiles[t][:, dm * 128:(dm + 1) * 128],
                        ctxT[t][:, qcb * qc:(qcb + 1) * qc],
                        start=(t == 0), stop=False,
                    )
                pend.append((dm, op))
            for dm, op in pend:
                close_dm(dm, op)

    nc.compile()
    return nc


def make_in_maps(Q, K, V, mask, Wq, bq, Wk, bk, Wv, bv, Wo):
    Q = np.asarray(Q, np.float32)
    K = np.asarray(K, np.float32)
    V = np.asarray(V, np.float32)
    mask = np.asarray(mask)
    n_do = DH // 128
    in_maps = []
    for c in range(N_CORES):
        b, hg = c // HG, c % HG
        cs = slice(hg * DH, (hg + 1) * DH)
        in_maps.append({
            "xqT": np.ascontiguousarray(Q[b].T).astype(np.float16),
            "xkT": np.ascontiguousarray(K[b].T).astype(np.float16),
            "xvT": np.ascontiguousarray(V[b].T).astype(np.float16),
            "maskT": np.ascontiguousarray(mask[b, 0].T).astype(np.float16),
            "wq": np.asarray(Wq, np.float32)[:, cs].astype(np.float16),
            "wk": np.asarray(Wk, np.float32)[:, cs].astype(np.float16),
            "wv": np.asarray(Wv, np.float32)[:, cs].astype(np.float16),
            "wo": np.asarray(Wo, np.float32)[cs, :].astype(np.float16),
            "bqT": np.ascontiguousarray(
                np.asarray(bq, np.float32)[cs].reshape(n_do, 128).T),
            "bkT": np.ascontiguousarray(
                np.asarray(bk, np.float32)[cs].reshape(n_do, 128).T),
            "bv": np.asarray(bv, np.float32)[cs].reshape(1, DH)
                .astype(ml_dtypes.bfloat16),
            "ones_d": np.ones((1, 512), ml_dtypes.bfloat16),
        })
    return in_maps


def combine_outputs(results, bo):
    out = np.empty((B, S, D), np.float32)
    for b in range(B):
        out[b] = (results[HG * b]["oT"].astype(np.float32)
                  + results[HG * b + 1]["oT"].astype(np.float32)).T
    out += np.asarray(bo, np.float32)
    return out


def kernel(Q, K, V, mask, Wq, bq, Wk, bk, Wv, bv, Wo, bo):
    from concourse.bass_utils import run_bass_kernel_spmd

    in_maps = make_in_maps(Q, K, V, mask, Wq, bq, Wk, bk, Wv, bv, Wo)
    nc = build_attention_nc()
    res = run_bass_kernel_spmd(nc, in_maps, core_ids=list(range(N_CORES)))
    return combine_outputs(res.results, bo)


# revision 66
# speedup vs baseline: 1.1876x; 1.1876x over previous
"""Multi-head attention Trainium2 kernel (v2).

Full inputs -> shard over 8 NeuronCores (batch x head-group) -> full output.

Per core c: batch b = c // 2, head-group hg = c % 2 (8 of 16 heads).
Column-shard Wq/Wk/Wv, row-shard Wo; each core computes a partial output
projection for its batch; host sums the two partials per batch and adds bo.

v2 layout/schedule (per core):
  - scores^T [k, q] with two heads of a pair packed on PE row halves
    (concurrent K=64 matmuls); exp on ScalarE only; mask multiply batched
    per kt-pair on DVE at 2x 16-bit rate.
  - V staged as [seq, 8 heads x 68] fp16 blocks: cols 0-63 = V, col 64 =
    ones (emits softmax denominators through the attn@V matmul), 65-67 pad
    for 4B alignment.  attn@V accumulates ctx^T rows 0-64 in PSUM.
  - normalization: reciprocal_approx_fast on PSUM row 64 -> gpsimd
    partition_broadcast -> one DVE multiply into ctxT.
  - schedule: K-proj, Q-proj, then attention starts; V-proj woven into
    q-chunk 0's score stream; output projection folded into each q-chunk.
  - PSUM: proj/scores pool 2x[128,1024] (4 banks) + ctx 2x[128,512]
    (2 banks) + out-proj 1x[128,1024] (2 banks) = 8 banks.
"""

import os
import sys

for _p in ("/opt/trn_rl_repo", "/root/.axon_site/_ro/trn_rl_repo"):
    if os.path.isdir(_p) and _p not in sys.path:
        sys.path.insert(0, _p)

import numpy as np
import ml_dtypes

B, S, D, H = 4, 2048, 1024, 16
DK = 64
N_CORES = 8
HG = 2                  # head groups (cores per batch)
DH = D // HG            # 512: d_out per core
QC = 512                # q-chunk width per score matmul (one PSUM bank)
VB = 68                 # va block stride (64 vals + ones col + 3 pad)


def build_attention_nc(s=S, d=D, dh=DH, qc=QC):
    """Build the single-core Bass program (SPMD across 8 cores)."""
    import concourse.mybir as mybir
    import concourse.tile as tile
    from concourse import bacc

    f32 = mybir.dt.float32
    f16 = mybir.dt.float16
    bf16 = mybir.dt.bfloat16
    EXPF = mybir.ActivationFunctionType.Exp

    n_h = dh // DK            # heads on this core (8)
    n_hp = n_h // 2           # head pairs (4)
    n_di = d // 128           # d_model 128-tiles (8)
    n_do = dh // 128          # d_out 128-tiles (4) == head pairs
    n_kt = s // 128           # key 128-tiles (16)
    n_qc = s // qc            # q chunks (4)
    n_st = s // 128           # seq 128-tiles (16)
    VA = n_h * VB             # va width per seq-tile

    nc = bacc.Bacc(None, target_bir_lowering=False)

    xqT = nc.dram_tensor("xqT", [d, s], f16, kind="ExternalInput")
    xkT = nc.dram_tensor("xkT", [d, s], f16, kind="ExternalInput")
    xvT = nc.dram_tensor("xvT", [d, s], f16, kind="ExternalInput")
    maskT = nc.dram_tensor("maskT", [s, s], f16, kind="ExternalInput")
    wq = nc.dram_tensor("wq", [d, dh], f16, kind="ExternalInput")
    wk = nc.dram_tensor("wk", [d, dh], f16, kind="ExternalInput")
    wv = nc.dram_tensor("wv", [d, dh], f16, kind="ExternalInput")
    wo = nc.dram_tensor("wo", [dh, d], f16, kind="ExternalInput")
    # bqT/bkT: [128, n_do] (column do = bias slice) for per-partition adds
    bqT = nc.dram_tensor("bqT", [128, n_do], f32, kind="ExternalInput")
    bkT = nc.dram_tensor("bkT", [128, n_do], f32, kind="ExternalInput")
    bv = nc.dram_tensor("bv", [1, dh], bf16, kind="ExternalInput")
    ones_d = nc.dram_tensor("ones_d", [1, 512], bf16, kind="ExternalInput")
    oT = nc.dram_tensor("oT", [d, s], f32, kind="ExternalOutput")

    # mask viewed as [p, kt, q] so one DMA grabs a [128, 8, qc] half-chunk
    maskT3 = maskT.rearrange("(kt p) q -> p kt q", p=128)

    scale = float(1.0 / np.sqrt(np.float32(DK)))

    with tile.TileContext(nc) as tc:
        with (
            tc.tile_pool(name="stage", bufs=16) as stage_pool,
            tc.tile_pool(name="w", bufs=16) as w_pool,
            tc.tile_pool(name="wo", bufs=n_do) as wo_pool,
            tc.tile_pool(name="qk", bufs=2 * n_do) as qk_pool,
            tc.tile_pool(name="va", bufs=n_st) as va_pool,
            tc.tile_pool(name="ctxT", bufs=n_do) as ctxT_pool,
            tc.tile_pool(name="mask", bufs=4) as mask_pool,
            tc.tile_pool(name="e", bufs=5) as e_pool,
            tc.tile_pool(name="nrm", bufs=1) as nrm_pool,
            tc.tile_pool(name="nrmbc", bufs=2) as nrmbc_pool,
            tc.tile_pool(name="osb", bufs=2) as osb_pool,
            tc.tile_pool(name="const", bufs=1) as const_pool,
            tc.tile_pool(name="ps", bufs=2, space="PSUM") as ps_pool,
            tc.tile_pool(name="cps", bufs=3, space="PSUM") as c_pool,
            tc.tile_pool(name="ops", bufs=1, space="PSUM") as o_pool,
        ):
            # ---------------- constants ---------------------------------
            ones = const_pool.tile([1, 128], bf16, tag="ones", name="ones")
            nc.sync.dma_start(ones[:, :], ones_d[:, 0:128])
            bqT_sb = const_pool.tile([128, n_do], f32, tag="biasq", name="bqT_sb")
            bkT_sb = const_pool.tile([128, n_do], f32, tag="biask", name="bkT_sb")
            bv_sb = const_pool.tile([1, dh], bf16, tag="biasv", name="bv_sb")
            nc.sync.dma_start(bqT_sb[:, :], bqT[:, :])
            nc.sync.dma_start(bkT_sb[:, :], bkT[:, :])
            nc.sync.dma_start(bv_sb[:, :], bv[:, :])

            # ---------------- input staging (pool-gated prefetch) -------
            def stage_x(xdram):
                xts = []
                for di in range(n_di):
                    xt = stage_pool.tile([128, s], f16, tag="x", name="xt")
                    nc.sync.dma_start(xt[:, :], xdram[di * 128:(di + 1) * 128, :])
                    xts.append(xt)
                return xts

            def stage_w(wdram):
                wts = []
                for di in range(n_di):
                    wt = w_pool.tile([128, dh], f16, tag="w", name="wt")
                    nc.sync.dma_start(wt[:, :], wdram[di * 128:(di + 1) * 128, :])
                    wts.append(wt)
                return wts

            # interleave wv/xv DMAs so V-proj's di-matmuls can start as soon
            # as each (wv[di], xv[di]) pair lands, not after the full wv set
            wv_t, xv_t = [], []
            for di in range(n_di):
                wt = w_pool.tile([128, dh], f16, tag="w", name="wt")
                nc.sync.dma_start(wt[:, :], wv[di * 128:(di + 1) * 128, :])
                wv_t.append(wt)
                xt = stage_pool.tile([128, s], f16, tag="x", name="xt")
                nc.sync.dma_start(xt[:, :], xvT[di * 128:(di + 1) * 128, :])
                xv_t.append(xt)
            wk_t = stage_w(wk)
            xk_t = stage_x(xkT)
            wq_t = stage_w(wq)
            xq_t = stage_x(xqT)
            wo_tiles = []
            for t in range(n_do):
                wt = wo_pool.tile([128, d], f16, tag="wo", name="wot")
                nc.sync.dma_start(wt[:, :], wo[t * 128:(t + 1) * 128, :])
                wo_tiles.append(wt)

            # ---------------- K / Q projections --------------------------
            def proj_kq(wts, xts, bsb, outs, do, scps=(0, 1)):
                ot = outs[do]
                for scp in scps:
                    ps = ps_pool.tile([128, 1024], f32, tag="ps", name="ps")
                    for half in range(2):
                        sc = scp * 2 + half
                        for di in range(n_di):
                            nc.tensor.matmul(
                                ps[:, half * 512:(half + 1) * 512],
                                wts[di][:, do * 128:(do + 1) * 128],
                                xts[di][:, sc * 512:(sc + 1) * 512],
                                start=(di == 0), stop=(di == n_di - 1),
                            )
                    nc.vector.tensor_scalar_add(
                        ot[:, scp * 1024:(scp + 1) * 1024],
                        ps[:, :], bsb[:, do:do + 1])

            kT = [qk_pool.tile([128, s], bf16, tag="qk", name=f"kT{t}")
                  for t in range(n_do)]
            qT = [qk_pool.tile([128, s], bf16, tag="qk", name=f"qT{t}")
                  for t in range(n_do)]

            # ---------------- V projection (emitted via closure) ---------
            va_tiles = [None] * n_st

            def emit_vproj(st_lo, st_hi):
                for stp in range(st_lo // 2, st_hi // 2):
                    vp = ps_pool.tile([128, 1024], f32, tag="ps", name="vp")
                    for half in range(2):
                        st = stp * 2 + half
                        for di in range(n_di):
                            nc.tensor.matmul(
                                vp[:, half * 512:(half + 1) * 512],
                                xv_t[di][:, st * 128:(st + 1) * 128],
                                wv_t[di][:, :],
                                start=(di == 0), stop=False,
                            )
                        nc.tensor.matmul(
                            vp[:, half * 512:(half + 1) * 512],
                            ones[:, 0:128], bv_sb[:, :],
                            start=False, stop=True,
                        )
                    for half in range(2):
                        st = stp * 2 + half
                        va = va_pool.tile([128, VA], f16, tag="va", name="va")
                        va3 = va.rearrange("p (h x) -> p h x", x=VB)
                        nc.vector.tensor_copy(
                            va3[:, :, 0:64],
                            vp[:, half * 512:(half + 1) * 512]
                            .rearrange("p (h x) -> p h x", x=64),
                        )
                        nc.gpsimd.memset(va3[:, :, 64:65], 1.0)
                        va_tiles[st] = va

            # ---------------- attention emission helpers -----------------
            mask_tiles = {}   # (qcb, quarter) -> tile [128, 4, qc]

            def emit_mask_dma(qcb):
                for quarter in range(4):
                    mt = mask_pool.tile([128, 4, qc], f16, tag="m", name="mt")
                    nc.sync.dma_start(
                        mt[:, :, :],
                        maskT3[:, quarter * 4:(quarter + 1) * 4,
                               qcb * qc:(qcb + 1) * qc],
                    )
                    mask_tiles[(qcb, quarter)] = mt

            pt_tiles = {}     # (qcb, hp, pair) -> masked-prob tile

            def emit_scores(qcb, hp, filler=None):
                for pair in range(n_kt // 2):
                    if pair == 2 and filler is not None:
                        filler()
                    et = e_pool.tile([128, 2 * 1024], f16, tag="e", name="et")
                    for j in range(2):
                        kt = pair * 2 + j
                        sp = ps_pool.tile([128, 1024], f32, tag="ps", name="sp")
                        for hh in range(2):
                            lo = hh * 64
                            nc.tensor.matmul(
                                sp[:, hh * qc:(hh + 1) * qc],
                                kT[hp][lo:lo + 64, kt * 128:(kt + 1) * 128],
                                qT[hp][lo:lo + 64, qcb * qc:(qcb + 1) * qc],
                                start=True, stop=True,
                            )
                        nc.scalar.activation(
                            et[:, j * 1024:(j + 1) * 1024], sp[:, :], EXPF,
                            scale=scale)
                    mt = mask_tiles[(qcb, pair // 2)]
                    m4 = (mt[:, (pair % 2) * 2:(pair % 2) * 2 + 2, :]
                          .unsqueeze(2).broadcast_to([128, 2, 2, qc]))
                    nc.vector.tensor_mul(
                        et[:, :].rearrange("p (k h q) -> p k h q", k=2, q=qc),
                        et[:, :].rearrange("p (k h q) -> p k h q", k=2, q=qc),
                        m4)
                    pt_tiles[(qcb, hp, pair)] = et

            def emit_attnv_norm(qcb, hp):
                cps = [c_pool.tile([128, qc], f32, tag="c", name="cp")
                       for _ in range(2)]
                for pair in range(n_kt // 2):
                    pt = pt_tiles.pop((qcb, hp, pair))
                    for j in range(2):
                        kt = pair * 2 + j
                        for hh in range(2):
                            h = hp * 2 + hh
                            nc.tensor.matmul(
                                cps[hh][0:65, :],
                                va_tiles[kt][:, h * VB:h * VB + 65],
                                pt[:, j * 1024 + hh * qc:
                                   j * 1024 + (hh + 1) * qc],
                                start=(kt == 0), stop=(kt == n_kt - 1),
                                skip_group_check=True,
                            )
                # normalize: ctx[0:64] * (1 / ctx[64]) -> ctxT
                for hh in range(2):
                    lt = nrm_pool.tile([1, qc], f32, tag="l", name="lt")
                    nc.vector.tensor_copy(lt[0:1, :], cps[hh][64:65, :])
                    rt = nrm_pool.tile([1, qc], f32, tag="r", name="rt")
                    nc.vector.reciprocal_approx_fast(rt[0:1, :], lt[0:1, :])
                    bc = nrmbc_pool.tile([64, qc], f32, tag="bc", name="bc")
                    nc.gpsimd.partition_broadcast(bc[:, :], rt[0:1, :], 64)
                    nc.vector.tensor_mul(
                        ctxT[hp][hh * 64:hh * 64 + 64, qcb * qc:(qcb + 1) * qc],
                        cps[hh][0:64, :], bc[:, :])

            def emit_outproj(qcb, dm_lo=0, dm_hi=None, pool=None):
                for dm in range(dm_lo, n_di if dm_hi is None else dm_hi):
                    op = (pool or o_pool).tile([128, 512], f32,
                                               tag="c" if pool else "o",
                                               name="ops")
                    for t in range(n_do):
                        nc.tensor.matmul(
                            op[:, :],
                            wo_tiles[t][:, dm * 128:(dm + 1) * 128],
                            ctxT[t][:, qcb * qc:(qcb + 1) * qc],
                            start=(t == 0), stop=(t == n_do - 1),
                        )
                    osb = osb_pool.tile([128, 512], f32, tag="osb", name="osb")
                    nc.vector.tensor_copy(osb[:, :], op[:, :])
                    nc.sync.dma_start(
                        oT[dm * 128:(dm + 1) * 128, qcb * qc:(qcb + 1) * qc],
                        osb[:, :],
                    )

            ctxT = [ctxT_pool.tile([128, s], f16, tag="ctxT", name=f"ctxT{t}")
                    for t in range(n_do)]

            # ---------------- schedule -----------------------------------
            # V-proj first, then K0/Q0 so attention streams early; the
            # remaining K/Q projections fill PE gaps during qcb0.
            emit_vproj(0, n_st)
            proj_kq(wk_t, xk_t, bkT_sb, kT, 0)
            proj_kq(wq_t, xq_t, bqT_sb, qT, 0)
            emit_mask_dma(0)
            emit_scores(0, 0)
            # qcb0 scores only read the scp0 half of qT, so Q[hp] scp1 is
            # deferred into qcb1's stream; K[hp] scp1 is woven between score
            # pairs (needed from pair 4 on) to avoid 14us projection blocks.
            for hp in range(1, n_hp):
                proj_kq(wk_t, xk_t, bkT_sb, kT, hp, scps=(0,))
                proj_kq(wq_t, xq_t, bqT_sb, qT, hp, scps=(0,))
                emit_attnv_norm(0, hp - 1)
                emit_scores(0, hp, filler=lambda h=hp: proj_kq(
                    wk_t, xk_t, bkT_sb, kT, h, scps=(1,)))
            emit_attnv_norm(0, n_hp - 1)
            # out-proj for qcb q is spread across qcb q+1's attention stream
            # (2 dm-groups per hp) so its single-bank evac stalls hide under
            # attention matmuls instead of idling the PE at qcb boundaries.
            for qcb in range(1, n_qc):
                emit_mask_dma(qcb)
                for hp in range(n_hp):
                    if qcb == 1 and hp >= 1:
                        proj_kq(wq_t, xq_t, bqT_sb, qT, hp, scps=(1,))
                    emit_scores(qcb, hp)
                    emit_outproj(qcb - 1, hp * 2, hp * 2 + 2)
                    emit_attnv_norm(qcb, hp)
            # final qcb's out-proj: attention is done, so rotate through the
            # (now idle) cps ring for pipelined evacuation.  Each dm group's
            # t=0..2 matmuls only need the first three head-pairs' ctxT, so
            # they run while the last head-pair's normalization drains; the
            # t=3 closer + evac follows once three groups are in flight.
            qcb = n_qc - 1

            def close_dm(dm, op):
                nc.tensor.matmul(
                    op[:, :],
                    wo_tiles[n_do - 1][:, dm * 128:(dm + 1) * 128],
                    ctxT[n_do - 1][:, qcb * qc:(qcb + 1) * qc],
                    start=False, stop=True,
                )
                osb = osb_pool.tile([128, 512], f32, tag="osb", name="osb")
                nc.vector.tensor_copy(osb[:, :], op[:, :])
                nc.sync.dma_start(
                    oT[dm * 128:(dm + 1) * 128, qcb * qc:(qcb + 1) * qc],
                    osb[:, :],
                )

            pend = []
            for dm in range(n_di):
                if len(pend) == 3:
                    close_dm(*pend.pop(0))
                # dm0 uses the free o-bank: its alloc would otherwise wait on
                # the cps slot released only by the last norm's multiply
                op = (o_pool if dm == 0 else c_pool).tile(
                    [128, 512], f32, tag="o" if dm == 0 else "c", name="ops")
                for t in range(n_do - 1):
                    nc.tensor.matmul(
                        op[:, :],
                        wo_tiles[t][:, dm * 128:(dm + 1) * 128],
                        ctxT[t][:, qcb * qc:(qcb + 1) * qc],
                        start=(t == 0), stop=False,
                    )
                pend.append((dm, op))
            for dm, op in pend:
                close_dm(dm, op)

    nc.compile()
    return nc


def make_in_maps(Q, K, V, mask, Wq, bq, Wk, bk, Wv, bv, Wo):
    Q = np.asarray(Q, np.float32)
    K = np.asarray(K, np.float32)
    V = np.asarray(V, np.float32)
    mask = np.asarray(mask)
    n_do = DH // 128
    in_maps = []
    for c in range(N_CORES):
        b, hg = c // HG, c % HG
        cs = slice(hg * DH, (hg + 1) * DH)
        in_maps.append({
            "xqT": np.ascontiguousarray(Q[b].T).astype(np.float16),
            "xkT": np.ascontiguousarray(K[b].T).astype(np.float16),
            "xvT": np.ascontiguousarray(V[b].T).astype(np.float16),
            "maskT": np.ascontiguousarray(mask[b, 0].T).astype(np.float16),
            "wq": np.asarray(Wq, np.float32)[:, cs].astype(np.float16),
            "wk": np.asarray(Wk, np.float32)[:, cs].astype(np.float16),
            "wv": np.asarray(Wv, np.float32)[:, cs].astype(np.float16),
            "wo": np.asarray(Wo, np.float32)[cs, :].astype(np.float16),
            "bqT": np.ascontiguousarray(
                np.asarray(bq, np.float32)[cs].reshape(n_do, 128).T),
            "bkT": np.ascontiguousarray(
                np.asarray(bk, np.float32)[cs].reshape(n_do, 128).T),
            "bv": np.asarray(bv, np.float32)[cs].reshape(1, DH)
                .astype(ml_dtypes.bfloat16),
            "ones_d": np.ones((1, 512), ml_dtypes.bfloat16),
        })
    return in_maps


def combine_outputs(results, bo):
    out = np.empty((B, S, D), np.float32)
    for b in range(B):
        out[b] = (results[HG * b]["oT"].astype(np.float32)
                  + results[HG * b + 1]["oT"].astype(np.float32)).T
    out += np.asarray(bo, np.float32)
    return out


def kernel(Q, K, V, mask, Wq, bq, Wk, bk, Wv, bv, Wo, bo):
    from concourse.bass_utils import run_bass_kernel_spmd

    in_maps = make_in_maps(Q, K, V, mask, Wq, bq, Wk, bk, Wv, bv, Wo)
    nc = build_attention_nc()
    res = run_bass_kernel_spmd(nc, in_maps, core_ids=list(range(N_CORES)))
    return combine_outputs(res.results, bo)
